# revision 5
# baseline (speedup 1.0000x reference)
"""
MinibatchDiscrimination kernel for 8x TRN2 NeuronCores (Bass/Tile).

Math:  x = inputs @ T  -> [B, K, D] with B=512, K=100, D=5
       out[a,k] = sum_b exp(-sum_d |x[a,k,d]-x[b,k,d]|)

v3 strategy (v2 was 49.8us; its engine trace showed ACT 83% / PE 79% /
DVE 69% all near-saturated by per-row fixed costs: per-row exp paid
~372ns of init+accum overhead, and the per-row -S identity matmul kept
PE at 5 matmuls/row):

  Pair coverage (as v2): core c owns global rows a = 64c+j (j=0..63) and
  window delta = 1..256 (partners b = a+delta mod 512). Deltas 1..255
  cover each unordered pair once; delta=256 pairs appear from both
  endpoints, and each endpoint keeps its own copy in its row sum while
  the cross path scatters only delta=1..255, so no correction columns
  are needed. The self term exp(0)=1 is added on the host.

  Factored-exp identity (replaces v2's exp-bias + identity-matmul):
    |u-v| = 2*relu(u-v) - u + v  =>  dist = 2R - S_a + S_b,
    exp(-dist) = exp(-2R) * exp(S_a) * exp(-S_b)
  with R = sum_d relu(x_a - x_b), S = sum_d x. The PSUM accumulates ONLY
  R (4 ones-matmuls per row, free size 256 - no identity matmul), the
  batched activation computes P = exp(-2R) <= 1 (safe in fp16, constant
  scale=-2, no bias, so it batches across rows), and a single DVE
  scalar_tensor_tensor rider per row applies both S factors AND the row
  sum:  dump = (P * G[k,j]) * E[k, j+window],  accum_out = sum(dump)
  where G = exp(S) (f32 per-partition scalars) and E = exp(-S) stored in
  BF16 (8-bit exponent: exp(+-12) cannot overflow, and its 0.4% mantissa
  error is far inside the tolerance).

  Per row j:
    DVE : relu chunks (2 or 3 of [125,256] fp16, 4x mode, 127ns)
    Pool: relu chunks (1 or 2)
    PE  : 4 d-sum matmuls ones[125,32] x ab -> R in a 4-bank-wide PSUM
          tile (fresh start/stop group per matmul)
    ACT : one exp(-2*R8) [128,2048] -> P8 (SBUF fp16) per EIGHT rows
    DVE : rider stt (P*G)*E -> dump16, accum_out -> raw32[:, j]
    Pool: cross[k, j+1..j+255] += dump16[k, 0:255]
  The rider/cross stream is emitted ~2 row-groups behind the
  relu/matmul stream so the in-order DVE/Pool queues never block on an
  exp; P8 and dump16 buffers rotate accordingly.

  dist psum layout: partition 32c+m holds k=25c+m (m<25); host
  reassembles own rows from raw32 and scatters cross columns t=1..318
  to rows (64c+t) % 512.
"""

import sys
import numpy as np

for _p in ("/opt/trn_rl_repo",):
    if _p not in sys.path:
        sys.path.insert(0, _p)

B = 512
F = 1024
K = 100
D = 5
KD = K * D  # 500
NCORES = 8
JPC = B // NCORES  # 64 output rows per core
NCHUNK = 4  # kd chunks of 125
CHUNK = KD // NCHUNK  # 125
KPC = K // NCHUNK  # 25 k's per chunk
FD = 256  # per-row window: delta = 1..256
W = JPC + FD  # 320 columns of x needed per core
GROUP = 8  # rows per exp batch
LAG = 2 * GROUP  # rider/cross emission lag (rows)

_NC_CACHE = {}


def build_nc():
    import contextlib

    import concourse.bass as bass
    import concourse.bacc as bacc
    import concourse.mybir as mybir
    from concourse.tile import TileContext

    nc = bacc.Bacc(None, target_bir_lowering=False, debug=True)

    inT = nc.declare_dram_parameter("inT", [F, W], mybir.dt.float16, isOutput=False)
    Tm = nc.declare_dram_parameter("Tm", [F, KD], mybir.dt.float16, isOutput=False)
    onesd = nc.declare_dram_parameter(
        "onesd", [CHUNK, 32], mybir.dt.float16, isOutput=False
    )
    raw_out = nc.declare_dram_parameter(
        "raw", [128, JPC], mybir.dt.float32, isOutput=True
    )
    cross_out = nc.declare_dram_parameter(
        "cross", [128, W], mybir.dt.float32, isOutput=True
    )

    with TileContext(nc) as tc:
        with tc.tile_pool(name="persist", bufs=1) as pp:
            T_sb = pp.tile([128, 8 * KD], mybir.dt.float16, name="T_sb")
            inT_sb = pp.tile([128, 8 * W], mybir.dt.float16, name="inT_sb")
            ones_sb = pp.tile([CHUNK, 32], mybir.dt.float16, name="ones_sb")
            xT_sb = pp.tile([128, NCHUNK * W], mybir.dt.float16, name="xT_sb")
            # f32 upcasts of xT columns 0..JPC (tensor_scalar per-partition
            # scalars must be f32)
            xTj_sb = pp.tile([128, NCHUNK * JPC], mybir.dt.float32, name="xTj_sb")
            E_sb = pp.tile([128, W], mybir.dt.bfloat16, name="E_sb")
            G_sb = pp.tile([128, JPC], mybir.dt.float32, name="G_sb")
            cross_sb = pp.tile([128, W], mybir.dt.float32, name="cross_sb")
            raw_sb = pp.tile([128, JPC], mybir.dt.float32, name="raw_sb")
            NP8 = 4
            p8_bufs = [
                pp.tile([128, GROUP * FD], mybir.dt.float16, name=f"p8_{i}")
                for i in range(NP8)
            ]
            NAB = 16
            ab_bufs = [
                pp.tile([CHUNK, FD], mybir.dt.float16, name=f"ab{i}")
                for i in range(NAB)
            ]
            NDP = 4
            dump_bufs = [
                pp.tile([128, FD], mybir.dt.float16, name=f"dp{i}")
                for i in range(NDP)
            ]

            # warm the ACT exp table while DMAs run (table load ~1.3us)
            warm_sb = pp.tile([1, 1], mybir.dt.float32, name="warm_sb")
            nc.vector.memset(warm_sb[:, :], 0.0)
            nc.scalar.activation(
                warm_sb[:, :], warm_sb[:, :], mybir.ActivationFunctionType.Exp
            )
            nc.vector.memset(cross_sb[:, :], 0.0)

            # --- load inputs: T quarters on the SP queue, inT quarters on
            # the ACT queue so descriptor generation overlaps ---
            for h in range(4):
                nc.sync.dma_start(
                    out=T_sb[:, h * 2 * KD : (h + 1) * 2 * KD],
                    in_=Tm[h * 256 : (h + 1) * 256, :].rearrange(
                        "(t p) c -> p t c", t=2
                    ),
                )
                nc.scalar.dma_start(
                    out=inT_sb[:, h * 2 * W : (h + 1) * 2 * W],
                    in_=inT[h * 256 : (h + 1) * 256, :].rearrange(
                        "(t p) c -> p t c", t=2
                    ),
                )
            nc.sync.dma_start(out=ones_sb[:, :], in_=onesd[:, :])

            psum_es = contextlib.ExitStack()
            psum = psum_es.enter_context(
                tc.tile_pool(name="psum", bufs=1, space="PSUM")
            )
            # Two 4-bank-wide dist tiles (all 8 psum banks). The projection
            # aliases its per-chunk accumulators into wide[0] (each chunk in
            # its own bank); S goes into wide[1] before the row loop claims
            # it.
            wide = [
                psum.tile([128, GROUP * FD], mybir.dt.float32, name=f"wide{i}")
                for i in range(2)
            ]
            xt_ps = [wide[0][0:CHUNK, c * 512 : c * 512 + W] for c in range(NCHUNK)]
            S_ps = wide[1][:, 0:W]
            # projection: t-outer for tiles 0..5 (runnable as DMA quarters
            # land), then per-chunk tails so each chunk's psum->sbuf copy
            # starts while the next chunk's tail matmuls run
            for t in range(6):
                for c in range(NCHUNK):
                    nc.tensor.matmul(
                        xt_ps[c],
                        T_sb[:, t * KD + c * CHUNK : t * KD + (c + 1) * CHUNK],
                        inT_sb[:, t * W : (t + 1) * W],
                        start=(t == 0),
                        stop=False,
                        skip_group_check=True,
                    )
            for c in range(NCHUNK):
                for t in (6, 7):
                    nc.tensor.matmul(
                        xt_ps[c],
                        T_sb[:, t * KD + c * CHUNK : t * KD + (c + 1) * CHUNK],
                        inT_sb[:, t * W : (t + 1) * W],
                        start=False,
                        stop=(t == 7),
                        skip_group_check=True,
                    )
                # psum->sbuf copies split between DVE and ACT
                eng = nc.vector.tensor_copy if c % 2 == 0 else nc.scalar.copy
                eng(xT_sb[0:CHUNK, c * W : (c + 1) * W], xt_ps[c])
                if c % 2 == 0:
                    nc.vector.tensor_copy(
                        xTj_sb[0:CHUNK, c * JPC : (c + 1) * JPC],
                        xT_sb[0:CHUNK, c * W : c * W + JPC],
                    )
                else:
                    nc.scalar.copy(
                        xTj_sb[0:CHUNK, c * JPC : (c + 1) * JPC],
                        xT_sb[0:CHUNK, c * W : c * W + JPC],
                    )
                # S[25c+m at partition 32c+m, i] = sum_d x[kd, i]
                nc.tensor.matmul(
                    S_ps[32 * c : 32 * c + 32, :],
                    ones_sb[:, :],
                    xT_sb[0:CHUNK, c * W : (c + 1) * W],
                    start=True,
                    stop=True,
                    tile_position=(0, 32 * c),
                )
            # dist = 2R' - S_b + S_a with R' = sum_d relu(x_b - x_a), so
            # dump = exp(-2R') * exp(S_b) * exp(-S_a):
            # E (window factor) = exp(+S) in bf16 (range-safe),
            # G (row scalar)   = exp(-S) in f32
            nc.scalar.activation(
                E_sb[:, :], S_ps[:, :], mybir.ActivationFunctionType.Exp,
                bias=0.0, scale=1.0,
            )
            nc.scalar.activation(
                G_sb[:, :], S_ps[:, 0:JPC], mybir.ActivationFunctionType.Exp,
                bias=0.0, scale=-1.0,
            )

            # --- main loop over output rows, software-pipelined: row j's
            # relu+matmul stream runs LAG rows ahead of the rider/cross
            # stream so DVE/Pool queues never wait on an exp ---
            def emit_front(j):
                g = j // GROUP
                jj = j % GROUP
                dist = wide[g % 2]
                for c in range(NCHUNK):
                    ab = ab_bufs[(j * NCHUNK + c) % NAB]
                    # Pool takes chunk 3 always and chunk 2 on even rows
                    on_pool = c == 3 or (c == 2 and j % 2 == 0)
                    eng = nc.gpsimd if on_pool else nc.vector
                    eng.tensor_scalar(
                        ab[:, :],
                        xT_sb[0:CHUNK, c * W + j + 1 : c * W + j + 1 + FD],
                        xTj_sb[0:CHUNK, c * JPC + j : c * JPC + j + 1],
                        0.0,
                        mybir.AluOpType.subtract,
                        mybir.AluOpType.max,
                    )
                    nc.tensor.matmul(
                        dist[32 * c : 32 * c + 32, jj * FD : (jj + 1) * FD],
                        ones_sb[:, :],
                        ab[:, :],
                        start=True,
                        stop=True,
                        tile_position=(0, 32 * c),
                        skip_group_check=True,
                    )
                if jj == GROUP - 1:
                    # batched P = exp(-2R) over the full 8-row group
                    nc.scalar.activation(
                        p8_bufs[g % NP8][:, :],
                        dist[:, :],
                        mybir.ActivationFunctionType.Exp,
                        bias=0.0,
                        scale=-2.0,
                    )

            def emit_rider(j):
                g = j // GROUP
                jj = j % GROUP
                p8 = p8_bufs[g % NP8]
                # dump = (P * exp(S_a)) * exp(-S_b);  accum = row sum
                nc.vector.scalar_tensor_tensor(
                    dump_bufs[j % NDP][:, :],
                    p8[:, jj * FD : (jj + 1) * FD],
                    G_sb[:, j : j + 1],
                    E_sb[:, j + 1 : j + 1 + FD],
                    mybir.AluOpType.mult,
                    mybir.AluOpType.mult,
                    accum_out=raw_sb[:, j : j + 1],
                )

            def emit_cross(j):
                # cross[k, j+delta] += dump[k, delta-1] for delta = 1..255
                # (delta=256 belongs to the partner row's own sum)
                nc.gpsimd.tensor_tensor(
                    cross_sb[:, j + 1 : j + FD],
                    cross_sb[:, j + 1 : j + FD],
                    dump_bufs[j % NDP][:, 0 : FD - 1],
                    mybir.AluOpType.add,
                )

            for j in range(JPC + LAG + 1):
                if j < JPC:
                    emit_front(j)
                if LAG <= j < JPC + LAG:
                    emit_rider(j - LAG)
                if j > LAG:
                    emit_cross(j - LAG - 1)

            psum_es.close()
            nc.scalar.dma_start(out=raw_out[:, :], in_=raw_sb[:, :])
            nc.sync.dma_start(out=cross_out[:, :], in_=cross_sb[:, :])

    nc.finalize()
    return nc


def _aux_consts():
    ob = np.zeros([CHUNK, 32], dtype=np.float16)
    for m in range(KPC):
        ob[5 * m : 5 * m + 5, m] = 1.0
    return ob


def make_in_maps(inputs, T):
    f16 = np.float16
    Tm = np.asarray(T, dtype=np.float32).astype(f16)
    ob = _aux_consts()
    in_maps = []
    for c in range(NCORES):
        rolled = np.roll(np.asarray(inputs, dtype=np.float32), -JPC * c, axis=0)
        inTc = np.ascontiguousarray(rolled[0:W].T).astype(f16)
        in_maps.append(
            {
                "inT": inTc,
                "Tm": Tm,
                "onesd": ob,
            }
        )
    return in_maps


def assemble_output(results):
    out = np.zeros([B, K], dtype=np.float32)
    for c in range(NCORES):
        rawc = np.asarray(results[c]["raw"], dtype=np.float32)  # [128, JPC]
        cross = np.asarray(results[c]["cross"], dtype=np.float32)  # [128, W]
        for cc in range(NCHUNK):
            ksl = slice(32 * cc, 32 * cc + KPC)
            kg = slice(KPC * cc, KPC * (cc + 1))
            # own rows: global rows 64c..64c+63 (+1.0 self term)
            out[JPC * c : JPC * (c + 1), kg] += rawc[ksl, :].T + 1.0
            # cross rows: global rows (64c + t) % 512 for t = 1..W-1
            rows = (JPC * c + np.arange(1, W)) % B
            np.add.at(
                out,
                (rows[:, None], np.arange(KPC * cc, KPC * (cc + 1))[None, :]),
                cross[ksl, 1:W].T,
            )
    return out


def kernel(inputs, T):
    from concourse.bass_utils import run_bass_kernel_spmd

    if "nc" not in _NC_CACHE:
        _NC_CACHE["nc"] = build_nc()
    nc = _NC_CACHE["nc"]
    in_maps = make_in_maps(inputs, T)
    res = run_bass_kernel_spmd(nc, in_maps, list(range(NCORES)))
    return assemble_output(res.results)


if __name__ == "__main__":
    sys.path.insert(0, "/root/problem")
    from reference import setup_inputs, reference

    inputs = setup_inputs()
    expected = np.asarray(reference(**inputs))
    actual = kernel(**{k: np.asarray(v) for k, v in inputs.items()})
    err = np.abs(actual - expected)
    rel = np.linalg.norm(actual - expected) / np.linalg.norm(expected)
    print(f"max abs err: {err.max():.3e}")
    print(f"Relative error: {rel:.3e}")


# revision 7
# speedup vs baseline: 1.0345x; 1.0345x over previous
"""
MinibatchDiscrimination kernel for 8x TRN2 NeuronCores (Bass/Tile).

Math:  x = inputs @ T  -> [B, K, D] with B=512, K=100, D=5
       out[a,k] = sum_b exp(-sum_d |x[a,k,d]-x[b,k,d]|)

v3 strategy (v2 was 49.8us; its engine trace showed ACT 83% / PE 79% /
DVE 69% all near-saturated by per-row fixed costs: per-row exp paid
~372ns of init+accum overhead, and the per-row -S identity matmul kept
PE at 5 matmuls/row):

  Pair coverage (as v2): core c owns global rows a = 64c+j (j=0..63) and
  window delta = 1..256 (partners b = a+delta mod 512). Deltas 1..255
  cover each unordered pair once; delta=256 pairs appear from both
  endpoints, and each endpoint keeps its own copy in its row sum while
  the cross path scatters only delta=1..255, so no correction columns
  are needed. The self term exp(0)=1 is added on the host.

  Factored-exp identity (replaces v2's exp-bias + identity-matmul):
    |u-v| = 2*relu(u-v) - u + v  =>  dist = 2R - S_a + S_b,
    exp(-dist) = exp(-2R) * exp(S_a) * exp(-S_b)
  with R = sum_d relu(x_a - x_b), S = sum_d x. The PSUM accumulates ONLY
  R (4 ones-matmuls per row, free size 256 - no identity matmul), the
  batched activation computes P = exp(-2R) <= 1 (safe in fp16, constant
  scale=-2, no bias, so it batches across rows), and a single DVE
  scalar_tensor_tensor rider per row applies both S factors AND the row
  sum:  dump = (P * G[k,j]) * E[k, j+window],  accum_out = sum(dump)
  where G = exp(S) (f32 per-partition scalars) and E = exp(-S) stored in
  BF16 (8-bit exponent: exp(+-12) cannot overflow, and its 0.4% mantissa
  error is far inside the tolerance).

  Per row j:
    DVE : relu chunks (2 or 3 of [125,256] fp16, 4x mode, 127ns)
    Pool: relu chunks (1 or 2)
    PE  : 4 d-sum matmuls ones[125,32] x ab -> R in a 4-bank-wide PSUM
          tile (fresh start/stop group per matmul)
    ACT : one exp(-2*R8) [128,2048] -> P8 (SBUF fp16) per EIGHT rows
    DVE : rider stt (P*G)*E -> dump16, accum_out -> raw32[:, j]
    Pool: cross[k, j+1..j+255] += dump16[k, 0:255]
  The rider/cross stream is emitted ~2 row-groups behind the
  relu/matmul stream so the in-order DVE/Pool queues never block on an
  exp; P8 and dump16 buffers rotate accordingly.

  dist psum layout: partition 32c+m holds k=25c+m (m<25); host
  reassembles own rows from raw32 and scatters cross columns t=1..318
  to rows (64c+t) % 512.
"""

import sys
import numpy as np

for _p in ("/opt/trn_rl_repo",):
    if _p not in sys.path:
        sys.path.insert(0, _p)

B = 512
F = 1024
K = 100
D = 5
KD = K * D  # 500
NCORES = 8
JPC = B // NCORES  # 64 output rows per core
NCHUNK = 4  # kd chunks of 125
CHUNK = KD // NCHUNK  # 125
KPC = K // NCHUNK  # 25 k's per chunk
FD = 256  # per-row window: delta = 1..256
W = JPC + FD  # 320 columns of x needed per core
GROUP = 8  # rows per exp batch
LAG = 2 * GROUP  # rider/cross emission lag (rows)

_NC_CACHE = {}


def build_nc():
    import contextlib

    import concourse.bass as bass
    import concourse.bacc as bacc
    import concourse.mybir as mybir
    from concourse.tile import TileContext

    nc = bacc.Bacc(None, target_bir_lowering=False, debug=True)

    inT = nc.declare_dram_parameter("inT", [F, W], mybir.dt.float16, isOutput=False)
    Tm = nc.declare_dram_parameter("Tm", [F, KD], mybir.dt.float16, isOutput=False)
    onesd = nc.declare_dram_parameter(
        "onesd", [CHUNK, 32], mybir.dt.float16, isOutput=False
    )
    raw_out = nc.declare_dram_parameter(
        "raw", [128, JPC], mybir.dt.float32, isOutput=True
    )
    cross_out = nc.declare_dram_parameter(
        "cross", [128, W], mybir.dt.float32, isOutput=True
    )

    with TileContext(nc) as tc:
        with tc.tile_pool(name="persist", bufs=1) as pp:
            T_sb = pp.tile([128, 8 * KD], mybir.dt.float16, name="T_sb")
            inT_sb = pp.tile([128, 8 * W], mybir.dt.float16, name="inT_sb")
            ones_sb = pp.tile([CHUNK, 32], mybir.dt.float16, name="ones_sb")
            xT_sb = pp.tile([128, NCHUNK * W], mybir.dt.float16, name="xT_sb")
            # f32 upcasts of xT columns 0..JPC (tensor_scalar per-partition
            # scalars must be f32)
            xTj_sb = pp.tile([128, NCHUNK * JPC], mybir.dt.float32, name="xTj_sb")
            E_sb = pp.tile([128, W], mybir.dt.bfloat16, name="E_sb")
            G_sb = pp.tile([128, JPC], mybir.dt.float32, name="G_sb")
            cross_sb = pp.tile([128, W], mybir.dt.float32, name="cross_sb")
            raw_sb = pp.tile([128, JPC], mybir.dt.float32, name="raw_sb")
            NP8 = 4
            p8_bufs = [
                pp.tile([128, GROUP * FD], mybir.dt.float16, name=f"p8_{i}")
                for i in range(NP8)
            ]
            NPE = 3
            pe8_bufs = [
                pp.tile([128, GROUP * FD], mybir.dt.float16, name=f"pe8_{i}")
                for i in range(NPE)
            ]
            NAB = 16
            ab_bufs = [
                pp.tile([CHUNK, FD], mybir.dt.float16, name=f"ab{i}")
                for i in range(NAB)
            ]
            NDP = 4
            dump_bufs = [
                pp.tile([128, FD], mybir.dt.float16, name=f"dp{i}")
                for i in range(NDP)
            ]

            # warm the ACT exp table while DMAs run (table load ~1.3us)
            warm_sb = pp.tile([1, 1], mybir.dt.float32, name="warm_sb")
            nc.vector.memset(warm_sb[:, :], 0.0)
            nc.scalar.activation(
                warm_sb[:, :], warm_sb[:, :], mybir.ActivationFunctionType.Exp
            )
            nc.vector.memset(cross_sb[:, :], 0.0)

            # --- load inputs: T quarters on the SP queue, inT quarters on
            # the ACT queue so descriptor generation overlaps ---
            for h in range(4):
                nc.sync.dma_start(
                    out=T_sb[:, h * 2 * KD : (h + 1) * 2 * KD],
                    in_=Tm[h * 256 : (h + 1) * 256, :].rearrange(
                        "(t p) c -> p t c", t=2
                    ),
                )
                nc.scalar.dma_start(
                    out=inT_sb[:, h * 2 * W : (h + 1) * 2 * W],
                    in_=inT[h * 256 : (h + 1) * 256, :].rearrange(
                        "(t p) c -> p t c", t=2
                    ),
                )
            nc.sync.dma_start(out=ones_sb[:, :], in_=onesd[:, :])

            psum_es = contextlib.ExitStack()
            psum = psum_es.enter_context(
                tc.tile_pool(name="psum", bufs=1, space="PSUM")
            )
            # Two 4-bank-wide dist tiles (all 8 psum banks). The projection
            # aliases its per-chunk accumulators into wide[0] (each chunk in
            # its own bank); S goes into wide[1] before the row loop claims
            # it.
            wide = [
                psum.tile([128, GROUP * FD], mybir.dt.float32, name=f"wide{i}")
                for i in range(2)
            ]
            xt_ps = [wide[0][0:CHUNK, c * 512 : c * 512 + W] for c in range(NCHUNK)]
            S_ps = wide[1][:, 0:W]
            # projection: t-outer for tiles 0..5 (runnable as DMA quarters
            # land), then per-chunk tails so each chunk's psum->sbuf copy
            # starts while the next chunk's tail matmuls run
            for t in range(6):
                for c in range(NCHUNK):
                    nc.tensor.matmul(
                        xt_ps[c],
                        T_sb[:, t * KD + c * CHUNK : t * KD + (c + 1) * CHUNK],
                        inT_sb[:, t * W : (t + 1) * W],
                        start=(t == 0),
                        stop=False,
                        skip_group_check=True,
                    )
            for c in range(NCHUNK):
                for t in (6, 7):
                    nc.tensor.matmul(
                        xt_ps[c],
                        T_sb[:, t * KD + c * CHUNK : t * KD + (c + 1) * CHUNK],
                        inT_sb[:, t * W : (t + 1) * W],
                        start=False,
                        stop=(t == 7),
                        skip_group_check=True,
                    )
                # psum->sbuf copies split between DVE and ACT
                eng = nc.vector.tensor_copy if c % 2 == 0 else nc.scalar.copy
                eng(xT_sb[0:CHUNK, c * W : (c + 1) * W], xt_ps[c])
                if c % 2 == 0:
                    nc.vector.tensor_copy(
                        xTj_sb[0:CHUNK, c * JPC : (c + 1) * JPC],
                        xT_sb[0:CHUNK, c * W : c * W + JPC],
                    )
                else:
                    nc.scalar.copy(
                        xTj_sb[0:CHUNK, c * JPC : (c + 1) * JPC],
                        xT_sb[0:CHUNK, c * W : c * W + JPC],
                    )
                # S[25c+m at partition 32c+m, i] = sum_d x[kd, i]
                nc.tensor.matmul(
                    S_ps[32 * c : 32 * c + 32, :],
                    ones_sb[:, :],
                    xT_sb[0:CHUNK, c * W : (c + 1) * W],
                    start=True,
                    stop=True,
                    tile_position=(0, 32 * c),
                )
            # dist = 2R' - S_b + S_a with R' = sum_d relu(x_b - x_a), so
            # dump = exp(-2R') * exp(S_b) * exp(-S_a):
            # E (window factor) = exp(+S) in bf16 (range-safe),
            # G (row scalar)   = exp(-S) in f32
            nc.scalar.activation(
                E_sb[:, :], S_ps[:, :], mybir.ActivationFunctionType.Exp,
                bias=0.0, scale=1.0,
            )
            nc.scalar.activation(
                G_sb[:, :], S_ps[:, 0:JPC], mybir.ActivationFunctionType.Exp,
                bias=0.0, scale=-1.0,
            )

            # --- main loop over output rows, software-pipelined: row j's
            # relu+matmul stream runs LAG rows ahead of the rider/cross
            # stream so DVE/Pool queues never wait on an exp ---
            import bass_rust

            def emit_front(j):
                g = j // GROUP
                jj = j % GROUP
                dist = wide[g % 2]
                for c in range(NCHUNK):
                    ab = ab_bufs[(j * NCHUNK + c) % NAB]
                    # Pool takes chunk 3 on odd rows only (~0.5 relus/row);
                    # DVE carries the rest
                    on_pool = c == 3 and j % 2 == 1
                    eng = nc.gpsimd if on_pool else nc.vector
                    eng.tensor_scalar(
                        ab[:, :],
                        xT_sb[0:CHUNK, c * W + j + 1 : c * W + j + 1 + FD],
                        xTj_sb[0:CHUNK, c * JPC + j : c * JPC + j + 1],
                        0.0,
                        mybir.AluOpType.subtract,
                        mybir.AluOpType.max,
                    )
                    nc.tensor.matmul(
                        dist[32 * c : 32 * c + 32, jj * FD : (jj + 1) * FD],
                        ones_sb[:, :],
                        ab[:, :],
                        start=True,
                        stop=True,
                        tile_position=(0, 32 * c),
                        skip_group_check=True,
                    )
                if jj == GROUP - 1:
                    # batched P = exp(-2R) over the full 8-row group
                    nc.scalar.activation(
                        p8_bufs[g % NP8][:, :],
                        dist[:, :],
                        mybir.ActivationFunctionType.Exp,
                        bias=0.0,
                        scale=-2.0,
                    )

            def emit_tt8(g):
                # one Pool multiply per group applies the E (= exp(S_b))
                # window factor to all 8 rows: in1 is an overlapping-window
                # AP over E (row r reads E[8g+r+1 .. 8g+r+256])
                ewin = E_sb[:, GROUP * g + 1 : GROUP * g + 1 + FD].copy()
                ewin.ap = bass_rust.VecI64Pair(
                    [tuple(ewin.ap[0]), (1, GROUP), (1, FD)]
                )
                nc.gpsimd.tensor_tensor(
                    pe8_bufs[g % NPE][:, :].rearrange("p (r c) -> p r c", r=GROUP),
                    p8_bufs[g % NP8][:, :].rearrange("p (r c) -> p r c", r=GROUP),
                    ewin,
                    mybir.AluOpType.mult,
                )

            def emit_rider(j):
                g = j // GROUP
                jj = j % GROUP
                pe8 = pe8_bufs[g % NPE]
                # dump = (P*E) * exp(-S_a);  accum_out = row sum (free on 4x)
                nc.vector.tensor_scalar(
                    dump_bufs[j % NDP][:, :],
                    pe8[:, jj * FD : (jj + 1) * FD],
                    G_sb[:, j : j + 1],
                    0.0,
                    mybir.AluOpType.mult,
                    mybir.AluOpType.add,
                    accum_out=raw_sb[:, j : j + 1],
                )

            def emit_cross(j):
                # cross[k, j+delta] += dump[k, delta-1] for delta = 1..255
                # (delta=256 belongs to the partner row's own sum)
                nc.gpsimd.tensor_tensor(
                    cross_sb[:, j + 1 : j + FD],
                    cross_sb[:, j + 1 : j + FD],
                    dump_bufs[j % NDP][:, 0 : FD - 1],
                    mybir.AluOpType.add,
                )

            for j in range(JPC + LAG + 1):
                if j < JPC:
                    emit_front(j)
                if j % GROUP == 0 and GROUP <= j < JPC + GROUP:
                    emit_tt8(j // GROUP - 1)
                if LAG <= j < JPC + LAG:
                    emit_rider(j - LAG)
                if j > LAG:
                    emit_cross(j - LAG - 1)

            psum_es.close()
            nc.scalar.dma_start(out=raw_out[:, :], in_=raw_sb[:, :])
            nc.sync.dma_start(out=cross_out[:, :], in_=cross_sb[:, :])

    nc.finalize()
    return nc


def _aux_consts():
    ob = np.zeros([CHUNK, 32], dtype=np.float16)
    for m in range(KPC):
        ob[5 * m : 5 * m + 5, m] = 1.0
    return ob


def make_in_maps(inputs, T):
    f16 = np.float16
    Tm = np.asarray(T, dtype=np.float32).astype(f16)
    ob = _aux_consts()
    in_maps = []
    for c in range(NCORES):
        rolled = np.roll(np.asarray(inputs, dtype=np.float32), -JPC * c, axis=0)
        inTc = np.ascontiguousarray(rolled[0:W].T).astype(f16)
        in_maps.append(
            {
                "inT": inTc,
                "Tm": Tm,
                "onesd": ob,
            }
        )
    return in_maps


def assemble_output(results):
    out = np.zeros([B, K], dtype=np.float32)
    for c in range(NCORES):
        rawc = np.asarray(results[c]["raw"], dtype=np.float32)  # [128, JPC]
        cross = np.asarray(results[c]["cross"], dtype=np.float32)  # [128, W]
        for cc in range(NCHUNK):
            ksl = slice(32 * cc, 32 * cc + KPC)
            kg = slice(KPC * cc, KPC * (cc + 1))
            # own rows: global rows 64c..64c+63 (+1.0 self term)
            out[JPC * c : JPC * (c + 1), kg] += rawc[ksl, :].T + 1.0
            # cross rows: global rows (64c + t) % 512 for t = 1..W-1
            rows = (JPC * c + np.arange(1, W)) % B
            np.add.at(
                out,
                (rows[:, None], np.arange(KPC * cc, KPC * (cc + 1))[None, :]),
                cross[ksl, 1:W].T,
            )
    return out


def kernel(inputs, T):
    from concourse.bass_utils import run_bass_kernel_spmd

    if "nc" not in _NC_CACHE:
        _NC_CACHE["nc"] = build_nc()
    nc = _NC_CACHE["nc"]
    in_maps = make_in_maps(inputs, T)
    res = run_bass_kernel_spmd(nc, in_maps, list(range(NCORES)))
    return assemble_output(res.results)


if __name__ == "__main__":
    sys.path.insert(0, "/root/problem")
    from reference import setup_inputs, reference

    inputs = setup_inputs()
    expected = np.asarray(reference(**inputs))
    actual = kernel(**{k: np.asarray(v) for k, v in inputs.items()})
    err = np.abs(actual - expected)
    rel = np.linalg.norm(actual - expected) / np.linalg.norm(expected)
    print(f"max abs err: {err.max():.3e}")
    print(f"Relative error: {rel:.3e}")


# revision 9
# speedup vs baseline: 1.0360x; 1.0014x over previous
"""
MinibatchDiscrimination kernel for 8x TRN2 NeuronCores (Bass/Tile).

Math:  x = inputs @ T  -> [B, K, D] with B=512, K=100, D=5
       out[a,k] = sum_b exp(-sum_d |x[a,k,d]-x[b,k,d]|)

v3 strategy (v2 was 49.8us; its engine trace showed ACT 83% / PE 79% /
DVE 69% all near-saturated by per-row fixed costs: per-row exp paid
~372ns of init+accum overhead, and the per-row -S identity matmul kept
PE at 5 matmuls/row):

  Pair coverage (as v2): core c owns global rows a = 64c+j (j=0..63) and
  window delta = 1..256 (partners b = a+delta mod 512). Deltas 1..255
  cover each unordered pair once; delta=256 pairs appear from both
  endpoints, and each endpoint keeps its own copy in its row sum while
  the cross path scatters only delta=1..255, so no correction columns
  are needed. The self term exp(0)=1 is added on the host.

  Factored-exp identity (replaces v2's exp-bias + identity-matmul):
    |u-v| = 2*relu(u-v) - u + v  =>  dist = 2R - S_a + S_b,
    exp(-dist) = exp(-2R) * exp(S_a) * exp(-S_b)
  with R = sum_d relu(x_a - x_b), S = sum_d x. The PSUM accumulates ONLY
  R (4 ones-matmuls per row, free size 256 - no identity matmul), the
  batched activation computes P = exp(-2R) <= 1 (safe in fp16, constant
  scale=-2, no bias, so it batches across rows), and a single DVE
  scalar_tensor_tensor rider per row applies both S factors AND the row
  sum:  dump = (P * G[k,j]) * E[k, j+window],  accum_out = sum(dump)
  where G = exp(S) (f32 per-partition scalars) and E = exp(-S) stored in
  BF16 (8-bit exponent: exp(+-12) cannot overflow, and its 0.4% mantissa
  error is far inside the tolerance).

  Per row j:
    DVE : relu chunks (2 or 3 of [125,256] fp16, 4x mode, 127ns)
    Pool: relu chunks (1 or 2)
    PE  : 4 d-sum matmuls ones[125,32] x ab -> R in a 4-bank-wide PSUM
          tile (fresh start/stop group per matmul)
    ACT : one exp(-2*R8) [128,2048] -> P8 (SBUF fp16) per EIGHT rows
    DVE : rider stt (P*G)*E -> dump16, accum_out -> raw32[:, j]
    Pool: cross[k, j+1..j+255] += dump16[k, 0:255]
  The rider/cross stream is emitted ~2 row-groups behind the
  relu/matmul stream so the in-order DVE/Pool queues never block on an
  exp; P8 and dump16 buffers rotate accordingly.

  dist psum layout: partition 32c+m holds k=25c+m (m<25); host
  reassembles own rows from raw32 and scatters cross columns t=1..318
  to rows (64c+t) % 512.
"""

import sys
import numpy as np

for _p in ("/opt/trn_rl_repo",):
    if _p not in sys.path:
        sys.path.insert(0, _p)

B = 512
F = 1024
K = 100
D = 5
KD = K * D  # 500
NCORES = 8
JPC = B // NCORES  # 64 output rows per core
NCHUNK = 4  # kd chunks of 125
CHUNK = KD // NCHUNK  # 125
KPC = K // NCHUNK  # 25 k's per chunk
FD = 256  # per-row window: delta = 1..256
W = JPC + FD  # 320 columns of x needed per core
GROUP = 8  # rows per exp batch
LAG = GROUP  # rider/cross emission lag (rows)

_NC_CACHE = {}


def build_nc():
    import contextlib

    import concourse.bass as bass
    import concourse.bacc as bacc
    import concourse.mybir as mybir
    from concourse.tile import TileContext

    nc = bacc.Bacc(None, target_bir_lowering=False, debug=True)

    inT = nc.declare_dram_parameter("inT", [F, W], mybir.dt.float16, isOutput=False)
    Tm = nc.declare_dram_parameter("Tm", [F, KD], mybir.dt.float16, isOutput=False)
    onesd = nc.declare_dram_parameter(
        "onesd", [CHUNK, 32], mybir.dt.float16, isOutput=False
    )
    raw_out = nc.declare_dram_parameter(
        "raw", [128, JPC], mybir.dt.float32, isOutput=True
    )
    cross_out = nc.declare_dram_parameter(
        "cross", [128, W], mybir.dt.float32, isOutput=True
    )

    with TileContext(nc) as tc:
        with tc.tile_pool(name="persist", bufs=1) as pp:
            T_sb = pp.tile([128, 8 * KD], mybir.dt.float16, name="T_sb")
            inT_sb = pp.tile([128, 8 * W], mybir.dt.float16, name="inT_sb")
            ones_sb = pp.tile([CHUNK, 32], mybir.dt.float16, name="ones_sb")
            xT_sb = pp.tile([128, NCHUNK * W], mybir.dt.float16, name="xT_sb")
            # f32 upcasts of xT columns 0..JPC (tensor_scalar per-partition
            # scalars must be f32)
            xTj_sb = pp.tile([128, NCHUNK * JPC], mybir.dt.float32, name="xTj_sb")
            E_sb = pp.tile([128, W], mybir.dt.bfloat16, name="E_sb")
            G_sb = pp.tile([128, JPC], mybir.dt.float32, name="G_sb")
            cross_sb = pp.tile([128, W], mybir.dt.float32, name="cross_sb")
            raw_sb = pp.tile([128, JPC], mybir.dt.float32, name="raw_sb")
            NP8 = 4
            p8_bufs = [
                pp.tile([128, GROUP * FD], mybir.dt.float16, name=f"p8_{i}")
                for i in range(NP8)
            ]
            NPE = 3
            pe8_bufs = [
                pp.tile([128, GROUP * FD], mybir.dt.float16, name=f"pe8_{i}")
                for i in range(NPE)
            ]
            NAB = 16
            ab_bufs = [
                pp.tile([CHUNK, FD], mybir.dt.float16, name=f"ab{i}")
                for i in range(NAB)
            ]
            NDP = 4
            dump_bufs = [
                pp.tile([128, FD], mybir.dt.float16, name=f"dp{i}")
                for i in range(NDP)
            ]

            # warm the ACT exp table while DMAs run (table load ~1.3us)
            warm_sb = pp.tile([1, 1], mybir.dt.float32, name="warm_sb")
            nc.vector.memset(warm_sb[:, :], 0.0)
            nc.scalar.activation(
                warm_sb[:, :], warm_sb[:, :], mybir.ActivationFunctionType.Exp
            )
            nc.vector.memset(cross_sb[:, :], 0.0)

            # --- load inputs: T quarters on the SP queue, inT quarters on
            # the ACT queue so descriptor generation overlaps ---
            for h in range(4):
                eng = nc.sync
                eng.dma_start(
                    out=T_sb[:, h * 2 * KD : (h + 1) * 2 * KD],
                    in_=Tm[h * 256 : (h + 1) * 256, :].rearrange(
                        "(t p) c -> p t c", t=2
                    ),
                )
                nc.scalar.dma_start(
                    out=inT_sb[:, h * 2 * W : (h + 1) * 2 * W],
                    in_=inT[h * 256 : (h + 1) * 256, :].rearrange(
                        "(t p) c -> p t c", t=2
                    ),
                )
            nc.sync.dma_start(out=ones_sb[:, :], in_=onesd[:, :])

            psum_es = contextlib.ExitStack()
            psum = psum_es.enter_context(
                tc.tile_pool(name="psum", bufs=1, space="PSUM")
            )
            # Two 4-bank-wide dist tiles (all 8 psum banks). The projection
            # aliases its per-chunk accumulators into wide[0] (each chunk in
            # its own bank); S goes into wide[1] before the row loop claims
            # it.
            wide = [
                psum.tile([128, GROUP * FD], mybir.dt.float32, name=f"wide{i}")
                for i in range(2)
            ]
            xt_ps = [wide[0][0:CHUNK, c * 512 : c * 512 + W] for c in range(NCHUNK)]
            S_ps = wide[1][:, 0:W]
            # projection: t-outer for tiles 0..5 (runnable as DMA quarters
            # land), then per-chunk tails so each chunk's psum->sbuf copy
            # starts while the next chunk's tail matmuls run
            for t in range(6):
                for c in range(NCHUNK):
                    nc.tensor.matmul(
                        xt_ps[c],
                        T_sb[:, t * KD + c * CHUNK : t * KD + (c + 1) * CHUNK],
                        inT_sb[:, t * W : (t + 1) * W],
                        start=(t == 0),
                        stop=False,
                        skip_group_check=True,
                    )
            for c in range(NCHUNK):
                for t in (6, 7):
                    nc.tensor.matmul(
                        xt_ps[c],
                        T_sb[:, t * KD + c * CHUNK : t * KD + (c + 1) * CHUNK],
                        inT_sb[:, t * W : (t + 1) * W],
                        start=False,
                        stop=(t == 7),
                        skip_group_check=True,
                    )
                # psum->sbuf copies split between DVE and ACT
                eng = nc.vector.tensor_copy if c % 2 == 0 else nc.scalar.copy
                eng(xT_sb[0:CHUNK, c * W : (c + 1) * W], xt_ps[c])
                if c % 2 == 0:
                    nc.vector.tensor_copy(
                        xTj_sb[0:CHUNK, c * JPC : (c + 1) * JPC],
                        xT_sb[0:CHUNK, c * W : c * W + JPC],
                    )
                else:
                    nc.scalar.copy(
                        xTj_sb[0:CHUNK, c * JPC : (c + 1) * JPC],
                        xT_sb[0:CHUNK, c * W : c * W + JPC],
                    )
                # S[25c+m at partition 32c+m, i] = sum_d x[kd, i]
                nc.tensor.matmul(
                    S_ps[32 * c : 32 * c + 32, :],
                    ones_sb[:, :],
                    xT_sb[0:CHUNK, c * W : (c + 1) * W],
                    start=True,
                    stop=True,
                    tile_position=(0, 32 * c),
                )
            # dist = 2R' - S_b + S_a with R' = sum_d relu(x_b - x_a), so
            # dump = exp(-2R') * exp(S_b) * exp(-S_a):
            # E (window factor) = exp(+S) in bf16 (range-safe),
            # G (row scalar)   = exp(-S) in f32
            nc.scalar.activation(
                E_sb[:, :], S_ps[:, :], mybir.ActivationFunctionType.Exp,
                bias=0.0, scale=1.0,
            )
            nc.scalar.activation(
                G_sb[:, :], S_ps[:, 0:JPC], mybir.ActivationFunctionType.Exp,
                bias=0.0, scale=-1.0,
            )

            # --- main loop over output rows, software-pipelined: row j's
            # relu+matmul stream runs LAG rows ahead of the rider/cross
            # stream so DVE/Pool queues never wait on an exp ---
            import bass_rust

            def emit_front(j):
                g = j // GROUP
                jj = j % GROUP
                dist = wide[g % 2]
                for c in range(NCHUNK):
                    ab = ab_bufs[(j * NCHUNK + c) % NAB]
                    # Pool takes chunk 3 on 5 of 8 rows (~0.625 relus/row);
                    # DVE carries the rest
                    on_pool = c == 3 and j % 8 in (1, 3, 5, 6, 7)
                    eng = nc.gpsimd if on_pool else nc.vector
                    eng.tensor_scalar(
                        ab[:, :],
                        xT_sb[0:CHUNK, c * W + j + 1 : c * W + j + 1 + FD],
                        xTj_sb[0:CHUNK, c * JPC + j : c * JPC + j + 1],
                        0.0,
                        mybir.AluOpType.subtract,
                        mybir.AluOpType.max,
                    )
                    nc.tensor.matmul(
                        dist[32 * c : 32 * c + 32, jj * FD : (jj + 1) * FD],
                        ones_sb[:, :],
                        ab[:, :],
                        start=True,
                        stop=True,
                        tile_position=(0, 32 * c),
                        skip_group_check=True,
                    )
                if jj == GROUP - 1:
                    # batched P = exp(-2R) over the full 8-row group
                    nc.scalar.activation(
                        p8_bufs[g % NP8][:, :],
                        dist[:, :],
                        mybir.ActivationFunctionType.Exp,
                        bias=0.0,
                        scale=-2.0,
                    )

            def emit_tt8(g):
                # one Pool multiply per group applies the E (= exp(S_b))
                # window factor to all 8 rows: in1 is an overlapping-window
                # AP over E (row r reads E[8g+r+1 .. 8g+r+256])
                ewin = E_sb[:, GROUP * g + 1 : GROUP * g + 1 + FD].copy()
                ewin.ap = bass_rust.VecI64Pair(
                    [tuple(ewin.ap[0]), (1, GROUP), (1, FD)]
                )
                nc.gpsimd.tensor_tensor(
                    pe8_bufs[g % NPE][:, :].rearrange("p (r c) -> p r c", r=GROUP),
                    p8_bufs[g % NP8][:, :].rearrange("p (r c) -> p r c", r=GROUP),
                    ewin,
                    mybir.AluOpType.mult,
                )

            def emit_rider(j):
                g = j // GROUP
                jj = j % GROUP
                pe8 = pe8_bufs[g % NPE]
                # dump = (P*E) * exp(-S_a);  accum_out = row sum (free on 4x)
                nc.vector.tensor_scalar(
                    dump_bufs[j % NDP][:, :],
                    pe8[:, jj * FD : (jj + 1) * FD],
                    G_sb[:, j : j + 1],
                    0.0,
                    mybir.AluOpType.mult,
                    mybir.AluOpType.add,
                    accum_out=raw_sb[:, j : j + 1],
                )

            def emit_cross(j):
                # cross[k, j+delta] += dump[k, delta-1] for delta = 1..255
                # (delta=256 belongs to the partner row's own sum)
                nc.gpsimd.tensor_tensor(
                    cross_sb[:, j + 1 : j + FD],
                    cross_sb[:, j + 1 : j + FD],
                    dump_bufs[j % NDP][:, 0 : FD - 1],
                    mybir.AluOpType.add,
                )

            for j in range(JPC + LAG + 1):
                if j < JPC:
                    emit_front(j)
                if j % GROUP == 0 and GROUP <= j < JPC + GROUP:
                    emit_tt8(j // GROUP - 1)
                if LAG <= j < JPC + LAG:
                    emit_rider(j - LAG)
                if j > LAG:
                    emit_cross(j - LAG - 1)

            psum_es.close()
            nc.scalar.dma_start(out=raw_out[:, :], in_=raw_sb[:, :])
            nc.sync.dma_start(out=cross_out[:, :], in_=cross_sb[:, :])

    nc.finalize()
    return nc


def _aux_consts():
    ob = np.zeros([CHUNK, 32], dtype=np.float16)
    for m in range(KPC):
        ob[5 * m : 5 * m + 5, m] = 1.0
    return ob


def make_in_maps(inputs, T):
    f16 = np.float16
    Tm = np.asarray(T, dtype=np.float32).astype(f16)
    ob = _aux_consts()
    in_maps = []
    for c in range(NCORES):
        rolled = np.roll(np.asarray(inputs, dtype=np.float32), -JPC * c, axis=0)
        inTc = np.ascontiguousarray(rolled[0:W].T).astype(f16)
        in_maps.append(
            {
                "inT": inTc,
                "Tm": Tm,
                "onesd": ob,
            }
        )
    return in_maps


def assemble_output(results):
    out = np.zeros([B, K], dtype=np.float32)
    for c in range(NCORES):
        rawc = np.asarray(results[c]["raw"], dtype=np.float32)  # [128, JPC]
        cross = np.asarray(results[c]["cross"], dtype=np.float32)  # [128, W]
        for cc in range(NCHUNK):
            ksl = slice(32 * cc, 32 * cc + KPC)
            kg = slice(KPC * cc, KPC * (cc + 1))
            # own rows: global rows 64c..64c+63 (+1.0 self term)
            out[JPC * c : JPC * (c + 1), kg] += rawc[ksl, :].T + 1.0
            # cross rows: global rows (64c + t) % 512 for t = 1..W-1
            rows = (JPC * c + np.arange(1, W)) % B
            np.add.at(
                out,
                (rows[:, None], np.arange(KPC * cc, KPC * (cc + 1))[None, :]),
                cross[ksl, 1:W].T,
            )
    return out


def kernel(inputs, T):
    from concourse.bass_utils import run_bass_kernel_spmd

    if "nc" not in _NC_CACHE:
        _NC_CACHE["nc"] = build_nc()
    nc = _NC_CACHE["nc"]
    in_maps = make_in_maps(inputs, T)
    res = run_bass_kernel_spmd(nc, in_maps, list(range(NCORES)))
    return assemble_output(res.results)


if __name__ == "__main__":
    sys.path.insert(0, "/root/problem")
    from reference import setup_inputs, reference

    inputs = setup_inputs()
    expected = np.asarray(reference(**inputs))
    actual = kernel(**{k: np.asarray(v) for k, v in inputs.items()})
    err = np.abs(actual - expected)
    rel = np.linalg.norm(actual - expected) / np.linalg.norm(expected)
    print(f"max abs err: {err.max():.3e}")
    print(f"Relative error: {rel:.3e}")
